# revision 33
# baseline (speedup 1.0000x reference)
"""Bounded attention (per-head QK RMSNorm + RoPE + KV-cache attention) on 8
Trainium2 NeuronCores.

Sharding: data parallel over batch. B=16 batches -> 2 per core; each core runs
all 16 heads over its own KV cache slice, no cross-core communication.

Per-core dataflow (fp16 K/V path, DMA-bound at ~94% DMA busy):
  - q/k/v (the 4 new positions) are staged through DRAM on the sync queue
    (rearranged to [(b h s), d]) and preprocessed (rmsnorm+rope fp32,
    PE-transpose, fp16) into qT16/kTn16 [d, (b,h,s)].
  - The KV cache streams via gpsimd casting DMAs: each [128 kv x 16h x 128d]
    row-group loads HBM fp32 -> SBUF fp16 with the cast done by the DMA
    engines (64 MiB of fp16 lands on-chip, 128 MiB read = the roofline).
  - Per tile: 16 fp16 PE transposes of K (2 PSUM banks, one DVE copy out),
    16 fp16 mm1 into one PSUM bank, one Exp on ACT ([128,64] -> fp16),
    16 fp16 mm2 + 1 sums matmul accumulating [d, (h q)] + colsums into one
    PSUM bank per batch. mm1 runs one tile behind the transposes and mm2 one
    behind mm1, so no engine waits on another within a tile.
  - The causal-masked 4x4 new-key corner (fp32) is folded in mid-stream;
    drain = 2 small transposes, reciprocal, scale, store via DRAM staging.
"""
import math
import numpy as np

import concourse.bass as bass
import concourse.tile as tile
from concourse import bacc, mybir
from concourse.bass_utils import run_bass_kernel_spmd

F32 = mybir.dt.float32
F16 = mybir.dt.float16
AF = mybir.ActivationFunctionType

B, S, DIM = 16, 4, 2048
H, D = 16, 128
KV = 4096
EPS = 1e-5
N_CORES = 8
B_LOC = B // N_CORES  # 2
TILES = KV // 128  # 32
SCALE = 1.0 / math.sqrt(D)
P = B_LOC * H * S  # 128 partitions in the (b, h, s) preproc layout

def _col(b, h):
    # column offset of (b, h)'s four queries in the qT/kT_new layouts
    return b * (H * S) + h * S


def _preprocess(nc, sb, pp, ps_pool, x_sb, w_sb, cos_sb, sin_sb, ident,
                eps_sb, name):
    """rmsnorm + rope of q or k, returns transposed fp16 [d, (b,h,s)] tile."""
    sq = pp.tile([P, D], F32, tag="pp_sq")
    ssq = pp.tile([P, 1], F32, tag=f"{name}_ssq")
    nc.scalar.activation(sq[:], x_sb[:], AF.Square, accum_out=ssq[:])
    std = pp.tile([P, 1], F32, tag=f"{name}_std")
    nc.scalar.activation(std[:], ssq[:], AF.Sqrt, bias=eps_sb[:],
                         scale=1.0 / D)
    rinv = pp.tile([P, 1], F32, tag=f"{name}_rinv")
    nc.vector.reciprocal(rinv[:], std[:])
    xn = pp.tile([P, D], F32, tag=f"{name}_xn")
    nc.vector.tensor_scalar_mul(xn[:], x_sb[:], rinv[:])
    xnw = pp.tile([P, D], F32, tag=f"{name}_xnw")
    nc.vector.tensor_mul(xnw[:], xn[:], w_sb[:])

    # rope on even/odd interleaved pairs
    xv = xnw[:].rearrange("p (x two) -> p x two", two=2)
    a, bb = xv[:, :, 0], xv[:, :, 1]
    xr = pp.tile([P, D], F32, tag=f"{name}_xr")
    xrv = xr[:].rearrange("p (x two) -> p x two", two=2)
    t1 = pp.tile([P, D // 2], F32, tag="pp_t1")
    t2 = pp.tile([P, D // 2], F32, tag="pp_t2")
    nc.vector.tensor_mul(t1[:], a, cos_sb[:])
    nc.vector.tensor_mul(t2[:], bb, sin_sb[:])
    nc.vector.tensor_sub(xrv[:, :, 0], t1[:], t2[:])
    t3 = pp.tile([P, D // 2], F32, tag="pp_t1")
    t4 = pp.tile([P, D // 2], F32, tag="pp_t2")
    nc.vector.tensor_mul(t3[:], a, sin_sb[:])
    nc.vector.tensor_mul(t4[:], bb, cos_sb[:])
    nc.vector.tensor_add(xrv[:, :, 1], t3[:], t4[:])

    # transpose -> [d, (b,h,s)], then fp16 copy to SBUF
    xT_ps = ps_pool.tile([D, 512], F32, tag="sT")
    nc.tensor.transpose(xT_ps[:, 0:P], xr[:], ident[:])
    xT16 = sb.tile([D, P], F16, tag=f"{name}_T16")
    nc.vector.tensor_copy(xT16[:], xT_ps[:, 0:P])
    return xT16


def build():
    nc = bacc.Bacc("TRN2", target_bir_lowering=False, debug=False,
                   num_devices=N_CORES)

    q_d = nc.dram_tensor("q", [B_LOC, S, DIM], F32, kind="ExternalInput").ap()
    k_d = nc.dram_tensor("k", [B_LOC, S, DIM], F32, kind="ExternalInput").ap()
    v_d = nc.dram_tensor("v", [B_LOC, S, DIM], F32, kind="ExternalInput").ap()
    ck_d = nc.dram_tensor("cache_k", [B_LOC, KV, H, D], F32,
                          kind="ExternalInput").ap()
    cv_d = nc.dram_tensor("cache_v", [B_LOC, KV, H, D], F32,
                          kind="ExternalInput").ap()
    cos_d = nc.dram_tensor("cos_b", [P, D // 2], F32, kind="ExternalInput").ap()
    sin_d = nc.dram_tensor("sin_b", [P, D // 2], F32, kind="ExternalInput").ap()
    wq_d = nc.dram_tensor("wq_b", [P, D], F32, kind="ExternalInput").ap()
    wk_d = nc.dram_tensor("wk_b", [P, D], F32, kind="ExternalInput").ap()
    id_d = nc.dram_tensor("ident", [128, 128], F32, kind="ExternalInput").ap()
    id16_d = nc.dram_tensor("ident16", [128, 128], F16,
                            kind="ExternalInput").ap()
    ones_d = nc.dram_tensor("ones", [128, 1], F32, kind="ExternalInput").ap()
    ones16_d = nc.dram_tensor("ones16", [128, 1], F16,
                              kind="ExternalInput").ap()
    mask_d = nc.dram_tensor("mask", [S, 16], F32, kind="ExternalInput").ap()
    out_d = nc.dram_tensor("out", [B_LOC, S, DIM], F32,
                           kind="ExternalOutput").ap()
    q_st = nc.dram_tensor("q_stage", [B_LOC, H, S, D], F32,
                          kind="Internal").ap()
    k_st = nc.dram_tensor("k_stage", [B_LOC, H, S, D], F32,
                          kind="Internal").ap()
    v_st = nc.dram_tensor("v_stage", [S, B_LOC, H * D], F32,
                          kind="Internal").ap()
    o_st = nc.dram_tensor("o_stage", [B_LOC, H, S, D], F32,
                          kind="Internal").ap()

    with tile.TileContext(nc) as tc:
        with (
            tc.tile_pool(name="consts", bufs=1) as consts,
            tc.tile_pool(name="pp", bufs=1) as pp,
            tc.tile_pool(name="sb", bufs=1) as sb,
            tc.tile_pool(name="k16p", bufs=6) as k16p,
            tc.tile_pool(name="v16p", bufs=8) as v16p,
            tc.tile_pool(name="kTp", bufs=4) as kTp,
            tc.tile_pool(name="expp", bufs=6) as expp,
            tc.tile_pool(name="drain", bufs=2) as drain,
            tc.tile_pool(name="ps_s", bufs=2, space=bass.MemorySpace.PSUM) as ps_s,
            tc.tile_pool(name="kTps", bufs=2, space=bass.MemorySpace.PSUM) as kTps,
            tc.tile_pool(name="psacc", bufs=1, space=bass.MemorySpace.PSUM) as psacc,
        ):
            ident = consts.tile([128, 128], F32)
            nc.sync.dma_start(ident[:], id_d)
            ident16 = consts.tile([128, 128], F16)
            nc.sync.dma_start(ident16[:], id16_d)
            ones32 = consts.tile([128, 1], F32)
            nc.sync.dma_start(ones32[:], ones_d)
            ones16 = consts.tile([128, 1], F16)
            nc.sync.dma_start(ones16[:], ones16_d)
            mask16 = consts.tile([S, 16], F32)
            nc.sync.dma_start(mask16[:], mask_d)
            cos_sb = consts.tile([P, D // 2], F32)
            nc.sync.dma_start(cos_sb[:], cos_d)
            sin_sb = consts.tile([P, D // 2], F32)
            nc.sync.dma_start(sin_sb[:], sin_d)
            wq_sb = consts.tile([P, D], F32)
            nc.sync.dma_start(wq_sb[:], wq_d)
            wk_sb = consts.tile([P, D], F32)
            nc.sync.dma_start(wk_sb[:], wk_d)
            eps_sb = consts.tile([P, 1], F32)
            nc.vector.memset(eps_sb[:], EPS)

            # q/k/v loads: rearrange through DRAM staging on the gpsimd
            # queue AHEAD of the cache stream so they don't starve behind
            # it, then plain 2D loads into SBUF
            for b in range(B_LOC):
                nc.sync.dma_start(
                    q_st[b], q_d[b].rearrange("s (h d) -> h s d", h=H))
                nc.sync.dma_start(
                    k_st[b], k_d[b].rearrange("s (h d) -> h s d", h=H))
                nc.sync.dma_start(v_st[:, b, :], v_d[b])
            q_sb = pp.tile([P, D], F32, tag="q_x")
            nc.sync.dma_start(q_sb[:], q_st.rearrange("b h s d -> (b h s) d"))
            k_sb = pp.tile([P, D], F32, tag="k_x")
            nc.sync.dma_start(k_sb[:], k_st.rearrange("b h s d -> (b h s) d"))
            # v_new as [s, (b h d)] so per-(b,h) slices start at partition 0
            v_sb = sb.tile([S, B_LOC * H * D], F32, tag="v_sb")
            nc.sync.dma_start(
                v_sb[:], v_st.rearrange("s b f -> s (b f)"))

            qT16 = _preprocess(nc, sb, pp, ps_s, q_sb, wq_sb, cos_sb,
                               sin_sb, ident, eps_sb, "q")
            kTn16 = _preprocess(nc, sb, pp, ps_s, k_sb, wk_sb, cos_sb,
                                sin_sb, ident, eps_sb, "k")

            # one accumulation bank per batch:
            # cols h*4..h*4+4 = oT[d, q] of head h; [0:1, 64+h*4+q] = sum_j exp
            accs = []
            for b in range(B_LOC):
                acc_t = psacc.tile([128, 512], F32, tag=f"acc{b}",
                                   name=f"acc{b}")
                accs.append(acc_t)

            def mm1_block(b, t, kT16, v16):
                """scores + exp for tile (b, t); returns mm2 work item."""
                sT = ps_s.tile([128, 512], F32, tag="sT")
                for h in range(H):
                    c = _col(b, h)
                    nc.tensor.matmul(
                        sT[:, h * S:(h + 1) * S],
                        kT16[:, h * D:(h + 1) * D], qT16[:, c:c + S],
                        start=(h == 0), stop=(h == H - 1),
                        skip_group_check=True)
                eT = expp.tile([128, H * S], F16, tag="eT")
                nc.scalar.activation(eT[:], sT[:, 0:H * S], AF.Exp,
                                     scale=SCALE)
                return (b, t, v16, eT)

            def mm2_block(b, t, v16, eT):
                first = (t == 0)
                last = (t == TILES - 1)
                acc = accs[b]
                for h in range(H):
                    nc.tensor.matmul(
                        acc[:, h * S:h * S + S], v16[:, h * D:(h + 1) * D],
                        eT[:, h * S:(h + 1) * S], start=(first and h == 0),
                        stop=False, skip_group_check=True)
                nc.tensor.matmul(acc[0:1, 64:128], ones16[:], eT[:],
                                 start=False, stop=last,
                                 skip_group_check=True)
                if t == TILES // 2:
                    new_keys_block(b)

            def new_keys_block(b):
                # the 4 new (current) keys, causal-masked, fp32
                acc = accs[b]
                for hg in range(H // 4):
                    hs4 = range(hg * 4, hg * 4 + 4)
                    sn = ps_s.tile([128, 512], F32, tag="sT", name="sn")
                    for j, h in enumerate(hs4):
                        c = _col(b, h)
                        nc.tensor.matmul(sn[0:S, j * S:(j + 1) * S],
                                         kTn16[:, c:c + S], qT16[:, c:c + S],
                                         start=(j == 0), stop=(j == 3),
                                         skip_group_check=True)
                    en = expp.tile([S, 16], F32, tag="en")
                    nc.scalar.activation(en[:], sn[0:S, 0:16], AF.Exp,
                                         scale=SCALE)
                    enm = expp.tile([S, 16], F32, tag="enm")
                    nc.vector.tensor_mul(enm[:], en[:], mask16[:])

                    for j, h in enumerate(hs4):
                        f0 = (b * H + h) * D
                        nc.tensor.matmul(acc[:, h * S:h * S + S],
                                         v_sb[:, f0:f0 + D],
                                         enm[:, j * S:(j + 1) * S],
                                         start=False, stop=False,
                                         skip_group_check=True)
                    nc.tensor.matmul(
                        acc[0:1, 64 + hg * 16:64 + hg * 16 + 16],
                        ones32[0:S], enm[:], start=False, stop=False,
                        skip_group_check=True)

            for b in range(B_LOC):
                pend1 = None  # tile awaiting mm1 (b, t, kT16, v16)
                pend2 = None  # tile awaiting mm2 (b, t, v16, eT)
                for t in range(TILES):
                    rows = slice(t * 128, (t + 1) * 128)
                    k16 = k16p.tile([128, H * D], F16, tag="k16")
                    nc.gpsimd.dma_start(
                        k16[:].rearrange("p (h d) -> p h d", h=H),
                        ck_d[b, rows])
                    v16 = v16p.tile([128, H * D], F16, tag="v16")
                    nc.gpsimd.dma_start(
                        v16[:].rearrange("p (h d) -> p h d", h=H),
                        cv_d[b, rows])

                    # per-head PE transposes (fp16) into 2 PSUM banks,
                    # one DVE copy out
                    kT_ps = kTps.tile([128, 2048], F16, tag="kTps")
                    for h in range(H):
                        nc.tensor.matmul(
                            kT_ps[:, h * D:(h + 1) * D],
                            k16[:, h * D:(h + 1) * D], ident16[:],
                            is_transpose=True, start=(h % 8 == 0),
                            stop=(h % 8 == 7), skip_group_check=True)
                    kT16 = kTp.tile([128, H * D], F16, tag="kT16")
                    nc.vector.tensor_copy(kT16[:], kT_ps[:])

                    if pend1 is not None:
                        nxt2 = mm1_block(*pend1)
                        if pend2 is not None:
                            mm2_block(*pend2)
                        pend2 = nxt2
                    pend1 = (b, t, kT16, v16)

                # drain the software pipeline
                if pend1 is not None:
                    nxt2 = mm1_block(*pend1)
                    if pend2 is not None:
                        mm2_block(*pend2)
                    mm2_block(*nxt2)
                acc = accs[b]

                # drain: transpose, normalize, store
                acc_sb = drain.tile([128, 128], F32, tag="acc_sb")
                nc.vector.tensor_copy(acc_sb[:, 0:64], acc[:, 0:64])
                nc.vector.tensor_copy(acc_sb[0:1, 64:128], acc[0:1, 64:128])
                o_ps = ps_s.tile([128, 512], F32, tag="sT")
                nc.tensor.transpose(o_ps[0:64, 0:128], acc_sb[:, 0:64],
                                    ident[:])
                sums_ps = ps_s.tile([128, 512], F32, tag="sT")
                nc.tensor.transpose(sums_ps[0:64, 0:1], acc_sb[0:1, 64:128],
                                    ident[0:1, 0:1])
                rs = drain.tile([64, 1], F32, tag="rs")
                nc.vector.reciprocal(rs[:], sums_ps[0:64, 0:1])
                o_norm = drain.tile([64, 128], F32, tag="o_norm")
                nc.vector.tensor_scalar_mul(o_norm[:], o_ps[0:64, 0:128],
                                            rs[:])
                nc.sync.dma_start(
                    o_st[b].rearrange("h s d -> (h s) d"), o_norm[:])
                nc.sync.dma_start(
                    out_d[b].rearrange("s (h d) -> h s d", h=H), o_st[b])

    nc.compile()
    return nc


_NC_CACHE = []


def _get_nc():
    if not _NC_CACHE:
        _NC_CACHE.append(build())
    return _NC_CACHE[0]


def make_in_maps(inputs):
    return _make_in_maps(**inputs)


def _make_in_maps(q, k, v, freqs_cos, freqs_sin, cache_k, cache_v, q_norm_w,
                  k_norm_w):
    q = np.asarray(q, dtype=np.float32)
    k = np.asarray(k, dtype=np.float32)
    v = np.asarray(v, dtype=np.float32)
    cache_k = np.asarray(cache_k, dtype=np.float32)
    cache_v = np.asarray(cache_v, dtype=np.float32)
    freqs_cos = np.asarray(freqs_cos, dtype=np.float32)
    freqs_sin = np.asarray(freqs_sin, dtype=np.float32)
    q_norm_w = np.asarray(q_norm_w, dtype=np.float32)
    k_norm_w = np.asarray(k_norm_w, dtype=np.float32)

    # host-side constant marshalling (layout helpers only)
    cos_b = np.ascontiguousarray(
        np.broadcast_to(freqs_cos[None, None], (B_LOC, H, S, D // 2))
        .reshape(P, D // 2))
    sin_b = np.ascontiguousarray(
        np.broadcast_to(freqs_sin[None, None], (B_LOC, H, S, D // 2))
        .reshape(P, D // 2))
    wq_b = np.ascontiguousarray(np.broadcast_to(q_norm_w[None, :], (P, D)))
    wk_b = np.ascontiguousarray(np.broadcast_to(k_norm_w[None, :], (P, D)))
    ident = np.eye(128, dtype=np.float32)
    ident16 = np.eye(128, dtype=np.float16)
    ones = np.ones((128, 1), dtype=np.float32)
    ones16 = np.ones((128, 1), dtype=np.float16)
    # mask[t, i] = 1 if query i attends new key t (i >= t)
    mask = np.ascontiguousarray(
        (np.arange(S)[None, :] >= np.arange(S)[:, None]).astype(np.float32))
    mask = np.ascontiguousarray(np.tile(mask, (1, 4)))  # [4, 16] for 4 heads

    in_maps = []
    for i in range(N_CORES):
        bs = slice(i * B_LOC, (i + 1) * B_LOC)
        in_maps.append({
            "q": np.ascontiguousarray(q[bs]),
            "k": np.ascontiguousarray(k[bs]),
            "v": np.ascontiguousarray(v[bs]),
            "cache_k": np.ascontiguousarray(cache_k[bs]),
            "cache_v": np.ascontiguousarray(cache_v[bs]),
            "cos_b": cos_b, "sin_b": sin_b, "wq_b": wq_b, "wk_b": wk_b,
            "ident": ident, "ident16": ident16, "ones": ones,
            "ones16": ones16, "mask": mask,
        })
    return in_maps


def run(q, k, v, freqs_cos, freqs_sin, cache_k, cache_v, q_norm_w, k_norm_w,
        trace=False, tmpdir=None):
    in_maps = _make_in_maps(q, k, v, freqs_cos, freqs_sin, cache_k, cache_v,
                            q_norm_w, k_norm_w)
    nc = _get_nc()
    res = run_bass_kernel_spmd(nc, in_maps, list(range(N_CORES)), trace=trace,
                               tmpdir=tmpdir)
    out = np.concatenate([res.results[i]["out"] for i in range(N_CORES)],
                         axis=0)
    return out.reshape(B, S, DIM), res


def kernel(q, k, v, freqs_cos, freqs_sin, cache_k, cache_v, q_norm_w,
           k_norm_w):
    out, _ = run(q, k, v, freqs_cos, freqs_sin, cache_k, cache_v, q_norm_w,
                 k_norm_w)
    return out


# revision 34
# speedup vs baseline: 1.0539x; 1.0539x over previous
"""Bounded attention (per-head QK RMSNorm + RoPE + KV-cache attention) on 8
Trainium2 NeuronCores.

Sharding: data parallel over batch. B=16 batches -> 2 per core; each core runs
all 16 heads over its own KV cache slice, no cross-core communication.

Per-core dataflow (fp16 K/V path, DMA-bound at ~94% DMA busy):
  - q/k/v (the 4 new positions) are staged through DRAM on the sync queue
    (rearranged to [(b h s), d]) and preprocessed (rmsnorm+rope fp32,
    PE-transpose, fp16) into qT16/kTn16 [d, (b,h,s)].
  - The KV cache streams via gpsimd casting DMAs: each [128 kv x 16h x 128d]
    row-group loads HBM fp32 -> SBUF fp16 with the cast done by the DMA
    engines (64 MiB of fp16 lands on-chip, 128 MiB read = the roofline).
  - Per tile: 16 fp16 PE transposes of K (2 PSUM banks, one DVE copy out),
    16 fp16 mm1 into one PSUM bank, one Exp on ACT ([128,64] -> fp16),
    16 fp16 mm2 + 1 sums matmul accumulating [d, (h q)] + colsums into one
    PSUM bank per batch. mm1 runs one tile behind the transposes and mm2 one
    behind mm1, so no engine waits on another within a tile.
  - The causal-masked 4x4 new-key corner (fp32) is folded in mid-stream;
    drain = 2 small transposes, reciprocal, scale, store via DRAM staging.
"""
import math
import numpy as np

import concourse.bass as bass
import concourse.tile as tile
from concourse import bacc, mybir
from concourse.bass_utils import run_bass_kernel_spmd

F32 = mybir.dt.float32
F16 = mybir.dt.float16
AF = mybir.ActivationFunctionType

B, S, DIM = 16, 4, 2048
H, D = 16, 128
KV = 4096
EPS = 1e-5
N_CORES = 8
B_LOC = B // N_CORES  # 2
TILES = KV // 128  # 32
SCALE = 1.0 / math.sqrt(D)
P = B_LOC * H * S  # 128 partitions in the (b, h, s) preproc layout

def _col(b, h):
    # column offset of (b, h)'s four queries in the qT/kT_new layouts
    return b * (H * S) + h * S


def _preprocess(nc, sb, pp, ps_pool, x_sb, w_sb, cos_sb, sin_sb, ident,
                eps_sb, name):
    """rmsnorm + rope of q or k, returns transposed fp16 [d, (b,h,s)] tile."""
    sq = pp.tile([P, D], F32, tag="pp_sq")
    ssq = pp.tile([P, 1], F32, tag=f"{name}_ssq")
    nc.scalar.activation(sq[:], x_sb[:], AF.Square, accum_out=ssq[:])
    std = pp.tile([P, 1], F32, tag=f"{name}_std")
    nc.scalar.activation(std[:], ssq[:], AF.Sqrt, bias=eps_sb[:],
                         scale=1.0 / D)
    rinv = pp.tile([P, 1], F32, tag=f"{name}_rinv")
    nc.vector.reciprocal(rinv[:], std[:])
    xn = pp.tile([P, D], F32, tag=f"{name}_xn")
    nc.vector.tensor_scalar_mul(xn[:], x_sb[:], rinv[:])
    xnw = pp.tile([P, D], F32, tag=f"{name}_xnw")
    nc.vector.tensor_mul(xnw[:], xn[:], w_sb[:])

    # rope on even/odd interleaved pairs
    xv = xnw[:].rearrange("p (x two) -> p x two", two=2)
    a, bb = xv[:, :, 0], xv[:, :, 1]
    xr = pp.tile([P, D], F32, tag=f"{name}_xr")
    xrv = xr[:].rearrange("p (x two) -> p x two", two=2)
    t1 = pp.tile([P, D // 2], F32, tag="pp_t1")
    t2 = pp.tile([P, D // 2], F32, tag="pp_t2")
    nc.vector.tensor_mul(t1[:], a, cos_sb[:])
    nc.vector.tensor_mul(t2[:], bb, sin_sb[:])
    nc.vector.tensor_sub(xrv[:, :, 0], t1[:], t2[:])
    t3 = pp.tile([P, D // 2], F32, tag="pp_t1")
    t4 = pp.tile([P, D // 2], F32, tag="pp_t2")
    nc.vector.tensor_mul(t3[:], a, sin_sb[:])
    nc.vector.tensor_mul(t4[:], bb, cos_sb[:])
    nc.vector.tensor_add(xrv[:, :, 1], t3[:], t4[:])

    # transpose -> [d, (b,h,s)], then fp16 copy to SBUF
    xT_ps = ps_pool.tile([D, 512], F32, tag="sT")
    nc.tensor.transpose(xT_ps[:, 0:P], xr[:], ident[:])
    xT16 = sb.tile([D, P], F16, tag=f"{name}_T16")
    nc.vector.tensor_copy(xT16[:], xT_ps[:, 0:P])
    return xT16


def build():
    nc = bacc.Bacc("TRN2", target_bir_lowering=False, debug=False,
                   num_devices=N_CORES)

    q_d = nc.dram_tensor("q", [B_LOC, S, DIM], F32, kind="ExternalInput").ap()
    k_d = nc.dram_tensor("k", [B_LOC, S, DIM], F32, kind="ExternalInput").ap()
    v_d = nc.dram_tensor("v", [B_LOC, S, DIM], F32, kind="ExternalInput").ap()
    ck_d = nc.dram_tensor("cache_k", [B_LOC, KV, H, D], F32,
                          kind="ExternalInput").ap()
    cv_d = nc.dram_tensor("cache_v", [B_LOC, KV, H, D], F32,
                          kind="ExternalInput").ap()
    cos_d = nc.dram_tensor("cos_b", [P, D // 2], F32, kind="ExternalInput").ap()
    sin_d = nc.dram_tensor("sin_b", [P, D // 2], F32, kind="ExternalInput").ap()
    wq_d = nc.dram_tensor("wq_b", [P, D], F32, kind="ExternalInput").ap()
    wk_d = nc.dram_tensor("wk_b", [P, D], F32, kind="ExternalInput").ap()
    id_d = nc.dram_tensor("ident", [128, 128], F32, kind="ExternalInput").ap()
    id16_d = nc.dram_tensor("ident16", [128, 128], F16,
                            kind="ExternalInput").ap()
    ones_d = nc.dram_tensor("ones", [128, 1], F32, kind="ExternalInput").ap()
    ones16_d = nc.dram_tensor("ones16", [128, 1], F16,
                              kind="ExternalInput").ap()
    mask_d = nc.dram_tensor("mask", [S, 16], F32, kind="ExternalInput").ap()
    out_d = nc.dram_tensor("out", [B_LOC, S, DIM], F32,
                           kind="ExternalOutput").ap()
    q_st = nc.dram_tensor("q_stage", [B_LOC, H, S, D], F32,
                          kind="Internal").ap()
    k_st = nc.dram_tensor("k_stage", [B_LOC, H, S, D], F32,
                          kind="Internal").ap()
    v_st = nc.dram_tensor("v_stage", [S, B_LOC, H * D], F32,
                          kind="Internal").ap()
    o_st = nc.dram_tensor("o_stage", [B_LOC, H, S, D], F32,
                          kind="Internal").ap()

    with tile.TileContext(nc) as tc:
        with (
            tc.tile_pool(name="consts", bufs=1) as consts,
            tc.tile_pool(name="pp", bufs=1) as pp,
            tc.tile_pool(name="sb", bufs=1) as sb,
            tc.tile_pool(name="k16p", bufs=6) as k16p,
            tc.tile_pool(name="v16p", bufs=8) as v16p,
            tc.tile_pool(name="kTp", bufs=4) as kTp,
            tc.tile_pool(name="expp", bufs=6) as expp,
            tc.tile_pool(name="drain", bufs=2) as drain,
            tc.tile_pool(name="ps_s", bufs=2, space=bass.MemorySpace.PSUM) as ps_s,
            tc.tile_pool(name="kTps", bufs=2, space=bass.MemorySpace.PSUM) as kTps,
            tc.tile_pool(name="psacc", bufs=1, space=bass.MemorySpace.PSUM) as psacc,
        ):
            ident = consts.tile([128, 128], F32)
            nc.sync.dma_start(ident[:], id_d)
            ident16 = consts.tile([128, 128], F16)
            nc.sync.dma_start(ident16[:], id16_d)
            ones32 = consts.tile([128, 1], F32)
            nc.sync.dma_start(ones32[:], ones_d)
            ones16 = consts.tile([128, 1], F16)
            nc.sync.dma_start(ones16[:], ones16_d)
            mask16 = consts.tile([S, 16], F32)
            nc.sync.dma_start(mask16[:], mask_d)
            cos_sb = consts.tile([P, D // 2], F32)
            nc.sync.dma_start(cos_sb[:], cos_d)
            sin_sb = consts.tile([P, D // 2], F32)
            nc.sync.dma_start(sin_sb[:], sin_d)
            wq_sb = consts.tile([P, D], F32)
            nc.sync.dma_start(wq_sb[:], wq_d)
            wk_sb = consts.tile([P, D], F32)
            nc.sync.dma_start(wk_sb[:], wk_d)
            eps_sb = consts.tile([P, 1], F32)
            nc.vector.memset(eps_sb[:], EPS)

            # q/k/v loads: rearrange through DRAM staging on the gpsimd
            # queue AHEAD of the cache stream so they don't starve behind
            # it, then plain 2D loads into SBUF
            for b in range(B_LOC):
                nc.sync.dma_start(
                    q_st[b], q_d[b].rearrange("s (h d) -> h s d", h=H))
                nc.sync.dma_start(
                    k_st[b], k_d[b].rearrange("s (h d) -> h s d", h=H))
                nc.sync.dma_start(v_st[:, b, :], v_d[b])
            q_sb = pp.tile([P, D], F32, tag="q_x")
            nc.sync.dma_start(q_sb[:], q_st.rearrange("b h s d -> (b h s) d"))
            k_sb = pp.tile([P, D], F32, tag="k_x")
            nc.sync.dma_start(k_sb[:], k_st.rearrange("b h s d -> (b h s) d"))
            # v_new as [s, (b h d)] so per-(b,h) slices start at partition 0
            v_sb = sb.tile([S, B_LOC * H * D], F32, tag="v_sb")
            nc.sync.dma_start(
                v_sb[:], v_st.rearrange("s b f -> s (b f)"))

            qT16 = _preprocess(nc, sb, pp, ps_s, q_sb, wq_sb, cos_sb,
                               sin_sb, ident, eps_sb, "q")
            kTn16 = _preprocess(nc, sb, pp, ps_s, k_sb, wk_sb, cos_sb,
                                sin_sb, ident, eps_sb, "k")

            # one accumulation bank per batch:
            # cols h*4..h*4+4 = oT[d, q] of head h; [0:1, 64+h*4+q] = sum_j exp
            accs = []
            for b in range(B_LOC):
                acc_t = psacc.tile([128, 512], F32, tag=f"acc{b}",
                                   name=f"acc{b}")
                accs.append(acc_t)

            def mm1_block(b, t, kT16, v16):
                """scores + exp for tile (b, t); returns mm2 work item."""
                sT = ps_s.tile([128, 512], F32, tag="sT")
                for h in range(H):
                    c = _col(b, h)
                    nc.tensor.matmul(
                        sT[:, h * S:(h + 1) * S],
                        kT16[:, h * D:(h + 1) * D], qT16[:, c:c + S],
                        start=(h == 0), stop=(h == H - 1),
                        skip_group_check=True)
                eT = expp.tile([128, H * S], F16, tag="eT")
                nc.scalar.activation(eT[:], sT[:, 0:H * S], AF.Exp,
                                     scale=SCALE)
                return (b, t, v16, eT)

            def mm2_block(b, t, v16, eT):
                first = (t == 0)
                last = (t == TILES - 1)
                acc = accs[b]
                for h in range(H):
                    nc.tensor.matmul(
                        acc[:, h * S:h * S + S], v16[:, h * D:(h + 1) * D],
                        eT[:, h * S:(h + 1) * S], start=(first and h == 0),
                        stop=False, skip_group_check=True)
                nc.tensor.matmul(acc[0:1, 64:128], ones16[:], eT[:],
                                 start=False, stop=last,
                                 skip_group_check=True)
                if first:
                    new_keys_block(b)

            def new_keys_block(b):
                # the 4 new (current) keys, causal-masked, fp32
                acc = accs[b]
                for hg in range(H // 4):
                    hs4 = range(hg * 4, hg * 4 + 4)
                    sn = ps_s.tile([128, 512], F32, tag="sT", name="sn")
                    for j, h in enumerate(hs4):
                        c = _col(b, h)
                        nc.tensor.matmul(sn[0:S, j * S:(j + 1) * S],
                                         kTn16[:, c:c + S], qT16[:, c:c + S],
                                         start=(j == 0), stop=(j == 3),
                                         skip_group_check=True)
                    en = expp.tile([S, 16], F32, tag="en")
                    nc.scalar.activation(en[:], sn[0:S, 0:16], AF.Exp,
                                         scale=SCALE)
                    enm = expp.tile([S, 16], F32, tag="enm")
                    nc.vector.tensor_mul(enm[:], en[:], mask16[:])

                    for j, h in enumerate(hs4):
                        f0 = (b * H + h) * D
                        nc.tensor.matmul(acc[:, h * S:h * S + S],
                                         v_sb[:, f0:f0 + D],
                                         enm[:, j * S:(j + 1) * S],
                                         start=False, stop=False,
                                         skip_group_check=True)
                    nc.tensor.matmul(
                        acc[0:1, 64 + hg * 16:64 + hg * 16 + 16],
                        ones32[0:S], enm[:], start=False, stop=False,
                        skip_group_check=True)

            for b in range(B_LOC):
                pend1 = None  # tile awaiting mm1 (b, t, kT16, v16)
                pend2 = None  # tile awaiting mm2 (b, t, v16, eT)
                for t in range(TILES):
                    rows = slice(t * 128, (t + 1) * 128)
                    k16 = k16p.tile([128, H * D], F16, tag="k16")
                    nc.gpsimd.dma_start(
                        k16[:].rearrange("p (h d) -> p h d", h=H),
                        ck_d[b, rows])
                    v16 = v16p.tile([128, H * D], F16, tag="v16")
                    nc.gpsimd.dma_start(
                        v16[:].rearrange("p (h d) -> p h d", h=H),
                        cv_d[b, rows])

                    # per-head PE transposes (fp16) into 2 PSUM banks,
                    # one DVE copy out
                    kT_ps = kTps.tile([128, 2048], F16, tag="kTps")
                    for h in range(H):
                        nc.tensor.matmul(
                            kT_ps[:, h * D:(h + 1) * D],
                            k16[:, h * D:(h + 1) * D], ident16[:],
                            is_transpose=True, start=(h % 8 == 0),
                            stop=(h % 8 == 7), skip_group_check=True)
                    kT16 = kTp.tile([128, H * D], F16, tag="kT16")
                    nc.vector.tensor_copy(kT16[:], kT_ps[:])

                    if pend1 is not None:
                        nxt2 = mm1_block(*pend1)
                        if pend2 is not None:
                            mm2_block(*pend2)
                        pend2 = nxt2
                    pend1 = (b, t, kT16, v16)

                # drain the software pipeline
                if pend1 is not None:
                    nxt2 = mm1_block(*pend1)
                    if pend2 is not None:
                        mm2_block(*pend2)
                    mm2_block(*nxt2)
                acc = accs[b]

                # drain: transpose, normalize, store
                acc_sb = drain.tile([128, 128], F32, tag="acc_sb")
                nc.vector.tensor_copy(acc_sb[:, 0:64], acc[:, 0:64])
                nc.vector.tensor_copy(acc_sb[0:1, 64:128], acc[0:1, 64:128])
                o_ps = ps_s.tile([128, 512], F32, tag="sT")
                nc.tensor.transpose(o_ps[0:64, 0:128], acc_sb[:, 0:64],
                                    ident[:])
                sums_ps = ps_s.tile([128, 512], F32, tag="sT")
                nc.tensor.transpose(sums_ps[0:64, 0:1], acc_sb[0:1, 64:128],
                                    ident[0:1, 0:1])
                rs = drain.tile([64, 1], F32, tag="rs")
                nc.vector.reciprocal(rs[:], sums_ps[0:64, 0:1])
                o_norm = drain.tile([64, 128], F32, tag="o_norm")
                nc.vector.tensor_scalar_mul(o_norm[:], o_ps[0:64, 0:128],
                                            rs[:])
                nc.sync.dma_start(
                    o_st[b].rearrange("h s d -> (h s) d"), o_norm[:])
                nc.sync.dma_start(
                    out_d[b].rearrange("s (h d) -> h s d", h=H), o_st[b])

    nc.compile()
    return nc


_NC_CACHE = []


def _get_nc():
    if not _NC_CACHE:
        _NC_CACHE.append(build())
    return _NC_CACHE[0]


def make_in_maps(inputs):
    return _make_in_maps(**inputs)


def _make_in_maps(q, k, v, freqs_cos, freqs_sin, cache_k, cache_v, q_norm_w,
                  k_norm_w):
    q = np.asarray(q, dtype=np.float32)
    k = np.asarray(k, dtype=np.float32)
    v = np.asarray(v, dtype=np.float32)
    cache_k = np.asarray(cache_k, dtype=np.float32)
    cache_v = np.asarray(cache_v, dtype=np.float32)
    freqs_cos = np.asarray(freqs_cos, dtype=np.float32)
    freqs_sin = np.asarray(freqs_sin, dtype=np.float32)
    q_norm_w = np.asarray(q_norm_w, dtype=np.float32)
    k_norm_w = np.asarray(k_norm_w, dtype=np.float32)

    # host-side constant marshalling (layout helpers only)
    cos_b = np.ascontiguousarray(
        np.broadcast_to(freqs_cos[None, None], (B_LOC, H, S, D // 2))
        .reshape(P, D // 2))
    sin_b = np.ascontiguousarray(
        np.broadcast_to(freqs_sin[None, None], (B_LOC, H, S, D // 2))
        .reshape(P, D // 2))
    wq_b = np.ascontiguousarray(np.broadcast_to(q_norm_w[None, :], (P, D)))
    wk_b = np.ascontiguousarray(np.broadcast_to(k_norm_w[None, :], (P, D)))
    ident = np.eye(128, dtype=np.float32)
    ident16 = np.eye(128, dtype=np.float16)
    ones = np.ones((128, 1), dtype=np.float32)
    ones16 = np.ones((128, 1), dtype=np.float16)
    # mask[t, i] = 1 if query i attends new key t (i >= t)
    mask = np.ascontiguousarray(
        (np.arange(S)[None, :] >= np.arange(S)[:, None]).astype(np.float32))
    mask = np.ascontiguousarray(np.tile(mask, (1, 4)))  # [4, 16] for 4 heads

    in_maps = []
    for i in range(N_CORES):
        bs = slice(i * B_LOC, (i + 1) * B_LOC)
        in_maps.append({
            "q": np.ascontiguousarray(q[bs]),
            "k": np.ascontiguousarray(k[bs]),
            "v": np.ascontiguousarray(v[bs]),
            "cache_k": np.ascontiguousarray(cache_k[bs]),
            "cache_v": np.ascontiguousarray(cache_v[bs]),
            "cos_b": cos_b, "sin_b": sin_b, "wq_b": wq_b, "wk_b": wk_b,
            "ident": ident, "ident16": ident16, "ones": ones,
            "ones16": ones16, "mask": mask,
        })
    return in_maps


def run(q, k, v, freqs_cos, freqs_sin, cache_k, cache_v, q_norm_w, k_norm_w,
        trace=False, tmpdir=None):
    in_maps = _make_in_maps(q, k, v, freqs_cos, freqs_sin, cache_k, cache_v,
                            q_norm_w, k_norm_w)
    nc = _get_nc()
    res = run_bass_kernel_spmd(nc, in_maps, list(range(N_CORES)), trace=trace,
                               tmpdir=tmpdir)
    out = np.concatenate([res.results[i]["out"] for i in range(N_CORES)],
                         axis=0)
    return out.reshape(B, S, DIM), res


def kernel(q, k, v, freqs_cos, freqs_sin, cache_k, cache_v, q_norm_w,
           k_norm_w):
    out, _ = run(q, k, v, freqs_cos, freqs_sin, cache_k, cache_v, q_norm_w,
                 k_norm_w)
    return out


# revision 35
# speedup vs baseline: 1.0675x; 1.0129x over previous
"""Bounded attention (per-head QK RMSNorm + RoPE + KV-cache attention) on 8
Trainium2 NeuronCores.

Sharding: data parallel over batch. B=16 batches -> 2 per core; each core runs
all 16 heads over its own KV cache slice, no cross-core communication.

Per-core dataflow (fp16 K/V path, DMA-bound at ~94% DMA busy):
  - q/k/v (the 4 new positions) are staged through DRAM on the sync queue
    (rearranged to [(b h s), d]) and preprocessed (rmsnorm+rope fp32,
    PE-transpose, fp16) into qT16/kTn16 [d, (b,h,s)].
  - The KV cache streams via gpsimd casting DMAs: each [128 kv x 16h x 128d]
    row-group loads HBM fp32 -> SBUF fp16 with the cast done by the DMA
    engines (64 MiB of fp16 lands on-chip, 128 MiB read = the roofline).
  - Per tile: 16 fp16 PE transposes of K (2 PSUM banks, one DVE copy out),
    16 fp16 mm1 into one PSUM bank, one Exp on ACT ([128,64] -> fp16),
    16 fp16 mm2 + 1 sums matmul accumulating [d, (h q)] + colsums into one
    PSUM bank per batch. mm1 runs one tile behind the transposes and mm2 one
    behind mm1, so no engine waits on another within a tile.
  - The causal-masked 4x4 new-key corner (fp32) is folded in mid-stream;
    drain = 2 small transposes, reciprocal, scale, store via DRAM staging.
"""
import math
import numpy as np

import concourse.bass as bass
import concourse.tile as tile
from concourse import bacc, mybir
from concourse.bass_utils import run_bass_kernel_spmd

F32 = mybir.dt.float32
F16 = mybir.dt.float16
AF = mybir.ActivationFunctionType

B, S, DIM = 16, 4, 2048
H, D = 16, 128
KV = 4096
EPS = 1e-5
N_CORES = 8
B_LOC = B // N_CORES  # 2
TILES = KV // 128  # 32
SCALE = 1.0 / math.sqrt(D)
P = B_LOC * H * S  # 128 partitions in the (b, h, s) preproc layout

def _col(b, h):
    # column offset of (b, h)'s four queries in the qT/kT_new layouts
    return b * (H * S) + h * S


def _preprocess(nc, sb, pp, ps_pool, x_sb, w_sb, cos_sb, sin_sb, ident,
                eps_sb, name):
    """rmsnorm + rope of q or k, returns transposed fp16 [d, (b,h,s)] tile."""
    sq = pp.tile([P, D], F32, tag="pp_sq")
    ssq = pp.tile([P, 1], F32, tag=f"{name}_ssq")
    nc.scalar.activation(sq[:], x_sb[:], AF.Square, accum_out=ssq[:])
    std = pp.tile([P, 1], F32, tag=f"{name}_std")
    nc.scalar.activation(std[:], ssq[:], AF.Sqrt, bias=eps_sb[:],
                         scale=1.0 / D)
    rinv = pp.tile([P, 1], F32, tag=f"{name}_rinv")
    nc.vector.reciprocal(rinv[:], std[:])
    xn = pp.tile([P, D], F32, tag=f"{name}_xn")
    nc.vector.tensor_scalar_mul(xn[:], x_sb[:], rinv[:])
    xnw = pp.tile([P, D], F32, tag=f"{name}_xnw")
    nc.vector.tensor_mul(xnw[:], xn[:], w_sb[:])

    # rope on even/odd interleaved pairs
    xv = xnw[:].rearrange("p (x two) -> p x two", two=2)
    a, bb = xv[:, :, 0], xv[:, :, 1]
    xr = pp.tile([P, D], F32, tag=f"{name}_xr")
    xrv = xr[:].rearrange("p (x two) -> p x two", two=2)
    t1 = pp.tile([P, D // 2], F32, tag="pp_t1")
    t2 = pp.tile([P, D // 2], F32, tag="pp_t2")
    nc.vector.tensor_mul(t1[:], a, cos_sb[:])
    nc.vector.tensor_mul(t2[:], bb, sin_sb[:])
    nc.vector.tensor_sub(xrv[:, :, 0], t1[:], t2[:])
    t3 = pp.tile([P, D // 2], F32, tag="pp_t1")
    t4 = pp.tile([P, D // 2], F32, tag="pp_t2")
    nc.vector.tensor_mul(t3[:], a, sin_sb[:])
    nc.vector.tensor_mul(t4[:], bb, cos_sb[:])
    nc.vector.tensor_add(xrv[:, :, 1], t3[:], t4[:])

    # transpose -> [d, (b,h,s)], then fp16 copy to SBUF
    xT_ps = ps_pool.tile([D, 512], F32, tag="sT")
    nc.tensor.transpose(xT_ps[:, 0:P], xr[:], ident[:])
    xT16 = sb.tile([D, P], F16, tag=f"{name}_T16")
    nc.vector.tensor_copy(xT16[:], xT_ps[:, 0:P])
    return xT16


def build():
    nc = bacc.Bacc("TRN2", target_bir_lowering=False, debug=False,
                   num_devices=N_CORES)

    q_d = nc.dram_tensor("q", [B_LOC, S, DIM], F32, kind="ExternalInput").ap()
    k_d = nc.dram_tensor("k", [B_LOC, S, DIM], F32, kind="ExternalInput").ap()
    v_d = nc.dram_tensor("v", [B_LOC, S, DIM], F32, kind="ExternalInput").ap()
    ck_d = nc.dram_tensor("cache_k", [B_LOC, KV, H, D], F32,
                          kind="ExternalInput").ap()
    cv_d = nc.dram_tensor("cache_v", [B_LOC, KV, H, D], F32,
                          kind="ExternalInput").ap()
    cos_d = nc.dram_tensor("cos_b", [P, D // 2], F32, kind="ExternalInput").ap()
    sin_d = nc.dram_tensor("sin_b", [P, D // 2], F32, kind="ExternalInput").ap()
    wq_d = nc.dram_tensor("wq_b", [P, D], F32, kind="ExternalInput").ap()
    wk_d = nc.dram_tensor("wk_b", [P, D], F32, kind="ExternalInput").ap()
    id_d = nc.dram_tensor("ident", [128, 128], F32, kind="ExternalInput").ap()
    id16_d = nc.dram_tensor("ident16", [128, 128], F16,
                            kind="ExternalInput").ap()
    ones_d = nc.dram_tensor("ones", [128, 1], F32, kind="ExternalInput").ap()
    ones16_d = nc.dram_tensor("ones16", [128, 1], F16,
                              kind="ExternalInput").ap()
    mask_d = nc.dram_tensor("mask", [S, 16], F32, kind="ExternalInput").ap()
    out_d = nc.dram_tensor("out", [B_LOC, S, DIM], F32,
                           kind="ExternalOutput").ap()
    q_st = nc.dram_tensor("q_stage", [B_LOC, H, S, D], F32,
                          kind="Internal").ap()
    k_st = nc.dram_tensor("k_stage", [B_LOC, H, S, D], F32,
                          kind="Internal").ap()
    v_st = nc.dram_tensor("v_stage", [S, B_LOC, H * D], F32,
                          kind="Internal").ap()
    o_st = nc.dram_tensor("o_stage", [B_LOC, H, S, D], F32,
                          kind="Internal").ap()

    with tile.TileContext(nc) as tc:
        with (
            tc.tile_pool(name="consts", bufs=1) as consts,
            tc.tile_pool(name="pp", bufs=1) as pp,
            tc.tile_pool(name="sb", bufs=1) as sb,
            tc.tile_pool(name="k16p", bufs=6) as k16p,
            tc.tile_pool(name="v16p", bufs=8) as v16p,
            tc.tile_pool(name="kTp", bufs=4) as kTp,
            tc.tile_pool(name="expp", bufs=6) as expp,
            tc.tile_pool(name="drain", bufs=2) as drain,
            tc.tile_pool(name="ps_s", bufs=2, space=bass.MemorySpace.PSUM) as ps_s,
            tc.tile_pool(name="kTps", bufs=2, space=bass.MemorySpace.PSUM) as kTps,
            tc.tile_pool(name="psacc", bufs=1, space=bass.MemorySpace.PSUM) as psacc,
        ):
            ident = consts.tile([128, 128], F32)
            nc.sync.dma_start(ident[:], id_d)
            ident16 = consts.tile([128, 128], F16)
            nc.sync.dma_start(ident16[:], id16_d)
            ones32 = consts.tile([128, 1], F32)
            nc.sync.dma_start(ones32[:], ones_d)
            ones16 = consts.tile([128, 1], F16)
            nc.sync.dma_start(ones16[:], ones16_d)
            mask16 = consts.tile([S, 16], F32)
            nc.sync.dma_start(mask16[:], mask_d)
            cos_sb = consts.tile([P, D // 2], F32)
            nc.sync.dma_start(cos_sb[:], cos_d)
            sin_sb = consts.tile([P, D // 2], F32)
            nc.sync.dma_start(sin_sb[:], sin_d)
            wq_sb = consts.tile([P, D], F32)
            nc.sync.dma_start(wq_sb[:], wq_d)
            wk_sb = consts.tile([P, D], F32)
            nc.sync.dma_start(wk_sb[:], wk_d)
            eps_sb = consts.tile([P, 1], F32)
            nc.vector.memset(eps_sb[:], EPS)

            # q/k/v loads: rearrange through DRAM staging on the gpsimd
            # queue AHEAD of the cache stream so they don't starve behind
            # it, then plain 2D loads into SBUF
            for b in range(B_LOC):
                nc.sync.dma_start(
                    q_st[b], q_d[b].rearrange("s (h d) -> h s d", h=H))
                nc.sync.dma_start(
                    k_st[b], k_d[b].rearrange("s (h d) -> h s d", h=H))
                nc.sync.dma_start(v_st[:, b, :], v_d[b])
            q_sb = pp.tile([P, D], F32, tag="q_x")
            nc.sync.dma_start(q_sb[:], q_st.rearrange("b h s d -> (b h s) d"))
            k_sb = pp.tile([P, D], F32, tag="k_x")
            nc.sync.dma_start(k_sb[:], k_st.rearrange("b h s d -> (b h s) d"))
            # v_new as [s, (b h d)] so per-(b,h) slices start at partition 0
            v_sb = sb.tile([S, B_LOC * H * D], F32, tag="v_sb")
            nc.sync.dma_start(
                v_sb[:], v_st.rearrange("s b f -> s (b f)"))

            qT16 = _preprocess(nc, sb, pp, ps_s, q_sb, wq_sb, cos_sb,
                               sin_sb, ident, eps_sb, "q")
            kTn16 = _preprocess(nc, sb, pp, ps_s, k_sb, wk_sb, cos_sb,
                                sin_sb, ident, eps_sb, "k")

            # one accumulation bank per batch:
            # cols h*4..h*4+4 = oT[d, q] of head h; [0:1, 64+h*4+q] = sum_j exp
            accs = []
            for b in range(B_LOC):
                acc_t = psacc.tile([128, 512], F32, tag=f"acc{b}",
                                   name=f"acc{b}")
                accs.append(acc_t)

            def mm1_block(b, t, kT16):
                """scores + exp for tile (b, t); returns mm2 work item."""
                sT = ps_s.tile([128, 512], F32, tag="sT")
                for h in range(H):
                    c = _col(b, h)
                    nc.tensor.matmul(
                        sT[:, h * S:(h + 1) * S],
                        kT16[:, h * D:(h + 1) * D], qT16[:, c:c + S],
                        start=(h == 0), stop=(h == H - 1),
                        skip_group_check=True)
                eT = expp.tile([128, H * S], F16, tag="eT")
                nc.scalar.activation(eT[:], sT[:, 0:H * S], AF.Exp,
                                     scale=SCALE)
                return (b, t, eT)

            def mm2_block(b, t, eT):
                first = (t == 0)
                last = (t == TILES - 1)
                v16 = v16s[t]
                acc = accs[b]
                for h in range(H):
                    nc.tensor.matmul(
                        acc[:, h * S:h * S + S], v16[:, h * D:(h + 1) * D],
                        eT[:, h * S:(h + 1) * S], start=(first and h == 0),
                        stop=False, skip_group_check=True)
                nc.tensor.matmul(acc[0:1, 64:128], ones16[:], eT[:],
                                 start=False, stop=last,
                                 skip_group_check=True)
                if first:
                    new_keys_block(b)

            def new_keys_block(b):
                # the 4 new (current) keys, causal-masked, fp32
                acc = accs[b]
                for hg in range(H // 4):
                    hs4 = range(hg * 4, hg * 4 + 4)
                    sn = ps_s.tile([128, 512], F32, tag="sT", name="sn")
                    for j, h in enumerate(hs4):
                        c = _col(b, h)
                        nc.tensor.matmul(sn[0:S, j * S:(j + 1) * S],
                                         kTn16[:, c:c + S], qT16[:, c:c + S],
                                         start=(j == 0), stop=(j == 3),
                                         skip_group_check=True)
                    en = expp.tile([S, 16], F32, tag="en")
                    nc.scalar.activation(en[:], sn[0:S, 0:16], AF.Exp,
                                         scale=SCALE)
                    enm = expp.tile([S, 16], F32, tag="enm")
                    nc.vector.tensor_mul(enm[:], en[:], mask16[:])

                    for j, h in enumerate(hs4):
                        f0 = (b * H + h) * D
                        nc.tensor.matmul(acc[:, h * S:h * S + S],
                                         v_sb[:, f0:f0 + D],
                                         enm[:, j * S:(j + 1) * S],
                                         start=False, stop=False,
                                         skip_group_check=True)
                    nc.tensor.matmul(
                        acc[0:1, 64 + hg * 16:64 + hg * 16 + 16],
                        ones32[0:S], enm[:], start=False, stop=False,
                        skip_group_check=True)

            def v_cast(b, t):
                v16 = v16p.tile([128, H * D], F16, tag="v16")
                nc.gpsimd.dma_start(
                    v16[:].rearrange("p (h d) -> p h d", h=H),
                    cv_d[b, t * 128:(t + 1) * 128])
                return v16

            for b in range(B_LOC):
                pend1 = None  # tile awaiting mm1 (b, t, kT16, v16)
                pend2 = None  # tile awaiting mm2 (b, t, v16, eT)
                v16s = {}
                for t in range(TILES):
                    rows = slice(t * 128, (t + 1) * 128)
                    k16 = k16p.tile([128, H * D], F16, tag="k16")
                    nc.gpsimd.dma_start(
                        k16[:].rearrange("p (h d) -> p h d", h=H),
                        ck_d[b, rows])
                    # v casts lag two tiles so the last k arrives early and
                    # the final tile's score chain overlaps the v tail
                    if t >= 2:
                        v16s[t - 2] = v_cast(b, t - 2)

                    # per-head PE transposes (fp16) into 2 PSUM banks,
                    # one DVE copy out
                    kT_ps = kTps.tile([128, 2048], F16, tag="kTps")
                    for h in range(H):
                        nc.tensor.matmul(
                            kT_ps[:, h * D:(h + 1) * D],
                            k16[:, h * D:(h + 1) * D], ident16[:],
                            is_transpose=True, start=(h % 8 == 0),
                            stop=(h % 8 == 7), skip_group_check=True)
                    kT16 = kTp.tile([128, H * D], F16, tag="kT16")
                    nc.vector.tensor_copy(kT16[:], kT_ps[:])

                    if pend1 is not None:
                        nxt2 = mm1_block(*pend1)
                        if pend2 is not None:
                            mm2_block(*pend2)
                        pend2 = nxt2
                    pend1 = (b, t, kT16)

                # flush the lagged v casts, then the software pipeline
                v16s[TILES - 2] = v_cast(b, TILES - 2)
                v16s[TILES - 1] = v_cast(b, TILES - 1)
                if pend1 is not None:
                    nxt2 = mm1_block(*pend1)
                    if pend2 is not None:
                        mm2_block(*pend2)
                    mm2_block(*nxt2)
                acc = accs[b]

                # drain: transpose, normalize, store
                acc_sb = drain.tile([128, 128], F32, tag="acc_sb")
                nc.vector.tensor_copy(acc_sb[:, 0:64], acc[:, 0:64])
                nc.vector.tensor_copy(acc_sb[0:1, 64:128], acc[0:1, 64:128])
                o_ps = ps_s.tile([128, 512], F32, tag="sT")
                nc.tensor.transpose(o_ps[0:64, 0:128], acc_sb[:, 0:64],
                                    ident[:])
                sums_ps = ps_s.tile([128, 512], F32, tag="sT")
                nc.tensor.transpose(sums_ps[0:64, 0:1], acc_sb[0:1, 64:128],
                                    ident[0:1, 0:1])
                rs = drain.tile([64, 1], F32, tag="rs")
                nc.vector.reciprocal(rs[:], sums_ps[0:64, 0:1])
                o_norm = drain.tile([64, 128], F32, tag="o_norm")
                nc.vector.tensor_scalar_mul(o_norm[:], o_ps[0:64, 0:128],
                                            rs[:])
                nc.sync.dma_start(
                    o_st[b].rearrange("h s d -> (h s) d"), o_norm[:])
                nc.sync.dma_start(
                    out_d[b].rearrange("s (h d) -> h s d", h=H), o_st[b])

    nc.compile()
    return nc


_NC_CACHE = []


def _get_nc():
    if not _NC_CACHE:
        _NC_CACHE.append(build())
    return _NC_CACHE[0]


def make_in_maps(inputs):
    return _make_in_maps(**inputs)


def _make_in_maps(q, k, v, freqs_cos, freqs_sin, cache_k, cache_v, q_norm_w,
                  k_norm_w):
    q = np.asarray(q, dtype=np.float32)
    k = np.asarray(k, dtype=np.float32)
    v = np.asarray(v, dtype=np.float32)
    cache_k = np.asarray(cache_k, dtype=np.float32)
    cache_v = np.asarray(cache_v, dtype=np.float32)
    freqs_cos = np.asarray(freqs_cos, dtype=np.float32)
    freqs_sin = np.asarray(freqs_sin, dtype=np.float32)
    q_norm_w = np.asarray(q_norm_w, dtype=np.float32)
    k_norm_w = np.asarray(k_norm_w, dtype=np.float32)

    # host-side constant marshalling (layout helpers only)
    cos_b = np.ascontiguousarray(
        np.broadcast_to(freqs_cos[None, None], (B_LOC, H, S, D // 2))
        .reshape(P, D // 2))
    sin_b = np.ascontiguousarray(
        np.broadcast_to(freqs_sin[None, None], (B_LOC, H, S, D // 2))
        .reshape(P, D // 2))
    wq_b = np.ascontiguousarray(np.broadcast_to(q_norm_w[None, :], (P, D)))
    wk_b = np.ascontiguousarray(np.broadcast_to(k_norm_w[None, :], (P, D)))
    ident = np.eye(128, dtype=np.float32)
    ident16 = np.eye(128, dtype=np.float16)
    ones = np.ones((128, 1), dtype=np.float32)
    ones16 = np.ones((128, 1), dtype=np.float16)
    # mask[t, i] = 1 if query i attends new key t (i >= t)
    mask = np.ascontiguousarray(
        (np.arange(S)[None, :] >= np.arange(S)[:, None]).astype(np.float32))
    mask = np.ascontiguousarray(np.tile(mask, (1, 4)))  # [4, 16] for 4 heads

    in_maps = []
    for i in range(N_CORES):
        bs = slice(i * B_LOC, (i + 1) * B_LOC)
        in_maps.append({
            "q": np.ascontiguousarray(q[bs]),
            "k": np.ascontiguousarray(k[bs]),
            "v": np.ascontiguousarray(v[bs]),
            "cache_k": np.ascontiguousarray(cache_k[bs]),
            "cache_v": np.ascontiguousarray(cache_v[bs]),
            "cos_b": cos_b, "sin_b": sin_b, "wq_b": wq_b, "wk_b": wk_b,
            "ident": ident, "ident16": ident16, "ones": ones,
            "ones16": ones16, "mask": mask,
        })
    return in_maps


def run(q, k, v, freqs_cos, freqs_sin, cache_k, cache_v, q_norm_w, k_norm_w,
        trace=False, tmpdir=None):
    in_maps = _make_in_maps(q, k, v, freqs_cos, freqs_sin, cache_k, cache_v,
                            q_norm_w, k_norm_w)
    nc = _get_nc()
    res = run_bass_kernel_spmd(nc, in_maps, list(range(N_CORES)), trace=trace,
                               tmpdir=tmpdir)
    out = np.concatenate([res.results[i]["out"] for i in range(N_CORES)],
                         axis=0)
    return out.reshape(B, S, DIM), res


def kernel(q, k, v, freqs_cos, freqs_sin, cache_k, cache_v, q_norm_w,
           k_norm_w):
    out, _ = run(q, k, v, freqs_cos, freqs_sin, cache_k, cache_v, q_norm_w,
                 k_norm_w)
    return out
